# revision 1
# baseline (speedup 1.0000x reference)
"""Trainium2 Bass kernel for nn_BlockContrastiveLoss.

Math: for x in [B*T, 16, 4], x_hat = x / max(||x||_block, 1e-12) per 4-dim
block. With S[v, 0:64] = segment_sum(x_hat) over the 1024-entry vocab and
S[v, 64] = counts c_v, the loss reduces exactly to

    loss = [ sum_v (||S_v||^2 - 16*c_v) / 32 ] / max(P, 1) * (P > 0)
    P    = 0.5 * (sum_v c_v^2 - sum_v c_v)

(c_v < 2 bins contribute exactly 0 to the numerator, so no mask is needed.)

Device strategy (8 cores, data-parallel over B*T per the sharding hint):
  - each core normalizes its 32768-token shard: ACT square/sqrt, GPSIMD
    pair-adds + reciprocal + scale (keeps the DVE free for the one-hot)
  - segment sum: per 128-token tile, ONE fp16 one-hot against a 512-wide
    iota (DVE tensor_scalar is_equal on t mod 512 -- halves the DVE work);
    the two vocab halves are routed by masking the stationary instead:
    two PE matmuls [x_hat*maskA | maskA]^T @ onehot512 -> PSUM[:, 0:512]
    and [x_hat*maskB | maskB]^T @ onehot512 -> PSUM[:, 512:1024],
    accumulated across all 256 tiles (S + counts land in PSUM [65, 1024]).
  - AllReduce the [65, 1024] table; every core computes the closed-form
    loss on-device; core 0's output is returned.
"""

import numpy as np

N_CORES = 8
B, T, D = 32, 8192, 64
V = 1024
TOK_PER_CORE = B * T // N_CORES      # 32768
P = 128
CHUNK_TOK = 2048                     # tokens per processing chunk
N_CHUNK = TOK_PER_CORE // CHUNK_TOK  # 16
J = CHUNK_TOK // P                   # 16 PE tiles (of 128 tokens) per chunk

_cache = {}
_opts = {"trace": False}


def _build_nc(single=False):
    import concourse.bacc as bacc
    import concourse.mybir as mybir
    import concourse.tile as tile

    dt = mybir.dt
    f32, f16, i32, i16 = dt.float32, dt.float16, dt.int32, dt.int16
    AF = mybir.ActivationFunctionType
    OP = mybir.AluOpType
    AX = mybir.AxisListType

    nc = bacc.Bacc("TRN2", target_bir_lowering=False, debug=False,
                   num_devices=1 if single else N_CORES)

    x_dram = nc.dram_tensor("x", [TOK_PER_CORE, D], f32, kind="ExternalInput")
    tok_dram = nc.dram_tensor("tok", [TOK_PER_CORE], i32, kind="ExternalInput")
    loss_dram = nc.dram_tensor("loss", [1, 1], f32, kind="ExternalOutput")
    cc_in = nc.dram_tensor("cc_in", [65, V], f32)
    cc_out = nc.dram_tensor("cc_out", [65, V], f32)

    with tile.TileContext(nc) as tc:
        with (
            tc.tile_pool(name="const", bufs=1) as constp,
            tc.tile_pool(name="xin", bufs=6) as xp,
            tc.tile_pool(name="mid", bufs=5) as midp,
            tc.tile_pool(name="oh", bufs=10) as ohp,
            tc.tile_pool(name="psum", bufs=1, space="PSUM") as psp,
            tc.tile_pool(name="ep", bufs=1) as epp,
        ):
            # ---- constants ----
            iota_i = constp.tile([P, V // 2], i16)
            nc.gpsimd.iota(iota_i[:], pattern=[[1, V // 2]], base=0,
                           channel_multiplier=0)
            iota_h = constp.tile([P, V // 2], f16)
            nc.vector.tensor_copy(iota_h[:], iota_i[:])

            # partition-major token order: partition p owns tokens
            # [p*256, (p+1)*256); chunk c tile j uses token p*256 + c*J + j.
            ids_i = constp.tile([P, N_CHUNK, J], i32)
            tok_v = tok_dram.ap().rearrange("(p n) -> p n", p=P)
            idsiF = ids_i[:].rearrange("p c j -> p (c j)")
            ids_h = constp.tile([P, N_CHUNK, J], f32)
            idsf = ids_h[:].rearrange("p c j -> p (c j)")
            # vocab-half split: compare against iota512 with t mod 512 and
            # route halves by masking the stationary operand per token.
            # The preamble is sliced so the first chunks' ids/masks are
            # ready quickly (shrinks the PE startup stall).
            NPPC = TOK_PER_CORE // P
            maskA = constp.tile([P, NPPC], f32)
            maskB = constp.tile([P, NPPC], f32)
            idsm = constp.tile([P, NPPC], f32)
            for s0, s1 in ((0, 16), (16, NPPC)):
                sl = slice(s0, s1)
                nc.sync.dma_start(idsiF[:, sl], tok_v[:, sl])
                nc.vector.tensor_copy(idsf[:, sl], idsiF[:, sl])
                nc.vector.tensor_scalar(maskA[:, sl], idsf[:, sl], 512.0,
                                        None, OP.is_lt)
                nc.vector.tensor_scalar(maskB[:, sl], idsf[:, sl], 512.0,
                                        None, OP.is_ge)
                nc.vector.tensor_scalar(idsm[:, sl], maskB[:, sl], 512.0,
                                        None, OP.mult)
                nc.vector.tensor_tensor(idsm[:, sl], idsf[:, sl],
                                        idsm[:, sl], OP.subtract)

            eps2 = constp.tile([P, 1], f32)
            nc.vector.memset(eps2[:], 1e-24)

            # preload the ACT function tables (Square / Abs_reciprocal_sqrt)
            # before the first chunk's chain needs them
            warm = constp.tile([P, 2], f32)
            nc.scalar.activation(warm[:, 0:1], eps2[:], AF.Square)
            nc.scalar.activation(warm[:, 1:2], eps2[:],
                                 AF.Abs_reciprocal_sqrt, bias=eps2[:])

            S_ps = psp.tile([65, V], f32)

            # per-partition token index n in [0, 256); chunks tile this range,
            # small chunks first so the PE pipeline fills quickly
            NPP = TOK_PER_CORE // P
            widths = [2, 2, 4, 8] + [16] * ((NPP - 16) // 16)
            assert sum(widths) == NPP
            x_n = x_dram.ap().rearrange("(p n) d -> p n d", p=P)

            n_off = 0
            for ci, W in enumerate(widths):
                xt = xp.tile([P, J * D], f32, tag="xt")
                xt = xt[:, 0:W * D].rearrange("p (j d) -> p j d", d=D)
                nc.sync.dma_start(xt, x_n[:, n_off:n_off + W, :])
                xt = xt.rearrange("p j d -> p (j d)")

                # squares (fp32 out) on ACT
                sq = midp.tile([P, J * D], f32, tag="sq")
                sq = sq[:, 0:W * D]
                nc.scalar.activation(sq, xt, AF.Square)

                # block norms^2: sum groups of 4 (GPSIMD)
                sq4 = sq.rearrange("p (t q) -> p t q", q=4)
                t2 = midp.tile([P, J * 16, 2], f32, tag="t2")
                t2 = t2[:, 0:W * 16, :]
                nc.vector.tensor_tensor(t2, sq4[:, :, 0:2], sq4[:, :, 2:4],
                                        OP.add)
                n2 = midp.tile([P, J * 16], f32, tag="n2")
                n2 = n2[:, 0:W * 16]
                nc.vector.tensor_tensor(n2, t2[:, :, 0], t2[:, :, 1], OP.add)

                # s = sqrt(n2 + eps^2) on ACT, inv ~ 1/s on DVE (1 op)
                s = midp.tile([P, J * 16], f32, tag="s")
                s = s[:, 0:W * 16]
                nc.scalar.activation(s, n2, AF.Sqrt, bias=eps2[:])
                inv = midp.tile([P, J * 16], f32, tag="inv")
                inv = inv[:, 0:W * 16]
                nc.vector.reciprocal_approx_fast(out=inv, in_=s)

                # masked inverse norms per vocab half (DVE: keeps the
                # producer chain on one engine; Pool does the big mults)
                mA = maskA[:, n_off:n_off + W].unsqueeze(2)
                mA = mA.broadcast_to([P, W, 16])
                mB = maskB[:, n_off:n_off + W].unsqueeze(2)
                mB = mB.broadcast_to([P, W, 16])
                invv = inv.rearrange("p (t b) -> p t b", b=16)
                invA = midp.tile([P, J * 16], f32, tag="invA")
                invA = invA[:, 0:W * 16].rearrange("p (t b) -> p t b", b=16)
                nc.vector.tensor_tensor(invA, invv, mA, OP.mult)
                invB = midp.tile([P, J * 16], f32, tag="invB")
                invB = invB[:, 0:W * 16].rearrange("p (t b) -> p t b", b=16)
                nc.vector.tensor_tensor(invB, invv, mB, OP.mult)

                # x_hat masked per half; col 64 carries the half's count mask
                xt4 = xt.rearrange("p (t b q) -> p t b q", b=16, q=4)
                xbA = xp.tile([P, J, 66], f16, tag="xbA")
                xbA = xbA[:, 0:W, :]
                nc.vector.tensor_copy(xbA[:, :, 64], maskA[:, n_off:n_off + W])
                xbA4 = xbA[:, :, 0:64].rearrange("p t (b q) -> p t b q", q=4)
                nc.gpsimd.tensor_tensor(
                    xbA4, xt4, invA.unsqueeze(3).broadcast_to([P, W, 16, 4]),
                    OP.mult)
                xbB = xp.tile([P, J, 66], f16, tag="xbB")
                xbB = xbB[:, 0:W, :]
                nc.vector.tensor_copy(xbB[:, :, 64], maskB[:, n_off:n_off + W])
                xbB4 = xbB[:, :, 0:64].rearrange("p t (b q) -> p t b q", q=4)
                nc.gpsimd.tensor_tensor(
                    xbB4, xt4, invB.unsqueeze(3).broadcast_to([P, W, 16, 4]),
                    OP.mult)

                for j in range(W):
                    oh = ohp.tile([P, V // 2], f16, tag="oh")
                    nc.vector.tensor_scalar(oh[:], iota_h[:],
                                            idsm[:, n_off + j:n_off + j + 1],
                                            None, OP.is_equal)
                    first = (ci == 0 and j == 0)
                    last = (ci == len(widths) - 1 and j == W - 1)
                    nc.tensor.matmul(S_ps[:, 0:512], xbA[:, j, 0:65], oh[:],
                                     start=first, stop=last)
                    nc.tensor.matmul(S_ps[:, 512:V], xbB[:, j, 0:65], oh[:],
                                     start=first, stop=last)
                n_off += W

            # ---- epilogue (half-sliced to pipeline copy/DMA/square) ----
            Sc = epp.tile([65, V], f32)
            for h in (0, 1):
                hs = slice(512 * h, 512 * (h + 1))
                nc.vector.tensor_copy(Sc[:, hs], S_ps[:, hs])
                nc.sync.dma_start(cc_in.ap()[:, hs], Sc[:, hs])
            if single:
                nc.sync.dma_start(cc_out.ap(), cc_in.ap())
            else:
                nc.gpsimd.collective_compute(
                    "AllReduce", OP.add,
                    replica_groups=[list(range(N_CORES))],
                    ins=[cc_in.ap().opt()], outs=[cc_out.ap().opt()],
                )
            R = epp.tile([65, V], f32)
            T2 = epp.tile([64, V], f32)
            ones64 = epp.tile([64, 1], f32)
            nc.vector.memset(ones64[:], 1.0)
            q_ps = psp.tile([1, V], f32)
            for h in (0, 1):
                hs = slice(512 * h, 512 * (h + 1))
                nc.sync.dma_start(R[:, hs], cc_out.ap()[:, hs])
                nc.scalar.activation(T2[:, hs], R[0:64, hs], AF.Square)
                nc.tensor.matmul(q_ps[:, hs], ones64[:], T2[:, hs],
                                 start=True, stop=True)

            c16 = epp.tile([1, V], f32)
            nc.vector.tensor_scalar(c16[:], R[64:65, :], 16.0, None, OP.mult)
            av = epp.tile([1, V], f32)
            nc.vector.tensor_tensor(av[:], q_ps[:], c16[:], OP.subtract)
            Ap = epp.tile([1, 1], f32)
            nc.vector.tensor_reduce(Ap[:], av[:], AX.X, OP.add)

            sumc = epp.tile([1, 1], f32)
            nc.vector.tensor_reduce(sumc[:], R[64:65, :], AX.X, OP.add)
            csq = epp.tile([1, V], f32)
            sumc2 = epp.tile([1, 1], f32)
            nc.scalar.activation(csq[:], R[64:65, :], AF.Square,
                                 accum_out=sumc2[:])

            pm = epp.tile([1, 1], f32)
            nc.vector.tensor_tensor(pm[:], sumc2[:], sumc[:], OP.subtract)
            nc.vector.tensor_scalar(pm[:], pm[:], 0.5, None, OP.mult)
            denom = epp.tile([1, 1], f32)
            nc.vector.tensor_scalar(denom[:], pm[:], 1.0, None, OP.max)
            maskp = epp.tile([1, 1], f32)
            nc.vector.tensor_scalar(maskp[:], pm[:], 0.0, None, OP.is_gt)
            numer = epp.tile([1, 1], f32)
            nc.vector.tensor_scalar(numer[:], Ap[:], 1.0 / 32.0, None, OP.mult)
            rden = epp.tile([1, 1], f32)
            nc.vector.reciprocal(rden[:], denom[:])
            lossv = epp.tile([1, 1], f32)
            nc.vector.tensor_tensor(lossv[:], numer[:], rden[:], OP.mult)
            nc.vector.tensor_tensor(lossv[:], lossv[:], maskp[:], OP.mult)
            nc.sync.dma_start(loss_dram.ap(), lossv[:])

    nc.compile()
    return nc


def kernel(semantic_state, token_ids):
    from concourse.bass_utils import run_bass_kernel_spmd

    if "nc" not in _cache:
        _cache["nc"] = _build_nc()
    nc = _cache["nc"]

    x = np.ascontiguousarray(np.asarray(semantic_state, dtype=np.float32)
                             ).reshape(N_CORES, TOK_PER_CORE, D)
    t = np.ascontiguousarray(np.asarray(token_ids).astype(np.int32)
                             ).reshape(N_CORES, TOK_PER_CORE)
    in_maps = [{"x": x[c], "tok": t[c]} for c in range(N_CORES)]
    res = run_bass_kernel_spmd(nc, in_maps, core_ids=list(range(N_CORES)),
                               trace=_opts["trace"])
    _cache["last_res"] = res
    out = np.asarray(res.results[0]["loss"], dtype=np.float32)
    return out.reshape(())



# revision 4
# speedup vs baseline: 1.5896x; 1.5896x over previous
"""Trainium2 Bass kernel for nn_BlockContrastiveLoss (fp8 DoubleRow design).

Math: for x in [B*T, 16, 4], x_hat = x / max(||x||_block, eps) per 4-dim
block. Let q = fp8e4m3(x_hat). The pairwise-cosine sum over each vocab
bin is computed EXACTLY for the quantized vectors via

    sum_{i<j in v} q_i . q_j = (||S_v||^2 - sum_{t in v} ||q_t||^2) / 2

so  numerator = (sum_v ||S_v||^2 - sum_t ||q_t||^2) / 32
    P         = (sum_v C_v^2 - N) / 2          (C = global counts)
    loss      = numerator / max(P, 1) * (P > 0)

The only approximation vs the fp32 reference is q != x_hat (measured
rel. err ~1.3e-2 against the jax oracle, within the 2e-2 gate).

Device strategy (8 cores, data-parallel over B*T):
  - partition-major layout: partition p owns tokens p*256+n, n in 0..256.
  - one-hot as packed fp16 words: word w of k-tile t is 56 (fp8 1.0 in
    low byte) if id==2w, 14336 (high byte) if id==2w+1 -- built in ONE
    DVE tensor_scalar (is_equal, mult) with two per-partition scalar
    pointers (s1 = id>>1, s2 = 56 + 14280*(id&1)). Bitcast to fp8 gives
    the full 1024-wide one-hot for 128 tokens in one ~194ns instruction.
  - fp8 DoubleRow matmuls contract 2 k-tiles (256 tokens) at once at
    0.5 cycles/column. Stationary = [q | 1.0 | pad] (80 cols; col 64
    folds the counts row; 65..79 keep preamble zeros to satisfy the
    dual-fp8 Ldweights step%16 rule). Three matmuls per pair:
    Gram [80,80] (trace = sum ||q||^2), S halves [80,512] x2.
  - AllReduce [66,1024]: rows 0..63 = S, 64 = counts, 65[0:64] = Gram
    diag partials; closed-form epilogue on-device; core 0 returns loss.
"""

import numpy as np

N_CORES = 8
B, T, D = 32, 8192, 64
V = 1024
TOK_PER_CORE = B * T // N_CORES      # 32768
P = 128
NPP = TOK_PER_CORE // P              # 256 tokens per partition (k-tiles)
NPAIR = NPP // 2                     # 128 DoubleRow pairs
NTOK_GLOBAL = float(B * T)

_cache = {}
_opts = {"trace": False}


def _build_nc(single=False):
    import concourse.bacc as bacc
    import concourse.mybir as mybir
    import concourse.tile as tile

    dt = mybir.dt
    f32, f16, i32, i16 = dt.float32, dt.float16, dt.int32, dt.int16
    f8 = dt.float8e4
    AF = mybir.ActivationFunctionType
    OP = mybir.AluOpType
    AX = mybir.AxisListType
    PM = mybir.MatmulPerfMode

    nc = bacc.Bacc("TRN2", target_bir_lowering=False, debug=False,
                   num_devices=1 if single else N_CORES)

    x_dram = nc.dram_tensor("x", [TOK_PER_CORE, D], f32, kind="ExternalInput")
    tok_dram = nc.dram_tensor("tok", [TOK_PER_CORE], i32, kind="ExternalInput")
    loss_dram = nc.dram_tensor("loss", [1, 1], f32, kind="ExternalOutput")
    cc_in = nc.dram_tensor("cc_in", [97, V], f32)
    cc_out = nc.dram_tensor("cc_out", [97, V], f32)

    with tile.TileContext(nc) as tc:
        with (
            tc.tile_pool(name="const", bufs=1) as constp,
            tc.tile_pool(name="xin", bufs=4) as xp,
            tc.tile_pool(name="mid", bufs=3) as midp,
            tc.tile_pool(name="oh", bufs=3) as ohp,
            tc.tile_pool(name="psum", bufs=1, space="PSUM") as psp,
            tc.tile_pool(name="ep", bufs=1) as epp,
        ):
            # ---- constants / preamble ----
            iota_i = constp.tile([P, V // 2], i16)
            nc.gpsimd.iota(iota_i[:], pattern=[[1, V // 2]], base=0,
                           channel_multiplier=0)

            ids_i = constp.tile([P, NPP], i32)
            tok_v = tok_dram.ap().rearrange("(p n) -> p n", p=P)
            s1 = constp.tile([P, NPP], f32)     # floor(id/2)
            s2 = constp.tile([P, NPP], f32)     # 56 + 14280*(id&1)
            s1i = constp.tile([P, NPP], i32)
            pari = constp.tile([P, NPP], i32)
            # sliced so the first pairs' scalars are ready quickly
            for a, b in ((0, 16), (16, 64), (64, NPP)):
                sl = slice(a, b)
                nc.sync.dma_start(ids_i[:, sl], tok_v[:, sl])
                nc.vector.tensor_scalar(s1i[:, sl], ids_i[:, sl], 1, None,
                                        OP.logical_shift_right)
                nc.vector.tensor_scalar(pari[:, sl], ids_i[:, sl], 1, None,
                                        OP.bitwise_and)
                nc.vector.tensor_copy(s1[:, sl], s1i[:, sl])
                nc.vector.tensor_copy(s2[:, sl], pari[:, sl])
                nc.vector.tensor_scalar(s2[:, sl], s2[:, sl], 14280.0, None,
                                        OP.mult)
                nc.vector.tensor_scalar(s2[:, sl], s2[:, sl], 56.0, None,
                                        OP.add)

            eps2 = constp.tile([P, 1], f32)
            nc.vector.memset(eps2[:], 1e-12)

            # preload ACT tables (Square / Abs_reciprocal_sqrt)
            warm = constp.tile([P, 2], f32)
            nc.scalar.activation(warm[:, 0:1], eps2[:], AF.Square)
            nc.scalar.activation(warm[:, 1:2], eps2[:],
                                 AF.Abs_reciprocal_sqrt, bias=eps2[:])

            # identity mask for the Gram-diag extraction (used in tail)
            ioc = constp.tile([64, 64], i16)
            nc.gpsimd.iota(ioc[:], pattern=[[1, 64]], base=0,
                           channel_multiplier=-1)
            idn = constp.tile([64, 64], f32)
            nc.vector.tensor_scalar(idn[:], ioc[:], 0.0, None, OP.is_equal)
            ones64 = constp.tile([64, 1], f32)
            nc.vector.memset(ones64[:], 1.0)

            # persistent x8 chunk buffers (manual rotation): [P, 16, 80] fp8
            # col 64 = 1.0 (counts row); cols 65..79 zeroed once, never
            # written again (the big-mult touches cols 0..63 only).
            NX8 = 3
            x8bufs = []
            for i in range(NX8):
                x8b = constp.tile([P, 16, 80], f8, tag=f"x8b{i}")
                nc.vector.memset(x8b[:, :, 64:80], 0.0)
                nc.vector.memset(x8b[:, :, 64], 1.0)
                x8bufs.append(x8b)

            # zero filler rows of the cc staging tile early (engine
            # partition bases must be 0/32/64/96, so the Gram-diag row
            # lives at partition 96; 65..95 stay zero)
            Sc = epp.tile([97, V], f32)
            nc.vector.memset(Sc[64:96, :], 0.0)
            nc.vector.memset(Sc[96:97, :], 0.0)

            # PSUM: S rows 0..63 = S, 64 = counts, 65..79 junk
            S_ps = psp.tile([80, V], f32)
            G_ps = psp.tile([80, 80], f32)

            x_n = x_dram.ap().rearrange("(p n) d -> p n d", p=P)

            widths = [2, 2, 4, 8] + [16] * ((NPP - 16) // 16)
            assert sum(widths) == NPP

            n_off = 0
            pair_idx = 0
            ci8 = 0
            for ci, W in enumerate(widths):
                # ---- x-side: load + normalize + quantize ----
                xt = xp.tile([P, 16, D], f32, tag="xt")
                xt = xt[:, 0:W, :]
                nc.sync.dma_start(xt, x_n[:, n_off:n_off + W, :])
                xtf = xt.rearrange("p j d -> p (j d)")

                sq = midp.tile([P, 16 * D], f16, tag="sq")
                sq = sq[:, 0:W * D]
                nc.scalar.activation(sq, xtf, AF.Square)

                sq4 = sq.rearrange("p (t q) -> p t q", q=4)
                t2 = midp.tile([P, 16 * 16, 2], f16, tag="t2")
                t2 = t2[:, 0:W * 16, :]
                nc.vector.tensor_tensor(t2, sq4[:, :, 0:2], sq4[:, :, 2:4],
                                        OP.add)
                n2 = midp.tile([P, 16 * 16], f16, tag="n2")
                n2 = n2[:, 0:W * 16]
                nc.gpsimd.tensor_tensor(n2, t2[:, :, 0], t2[:, :, 1], OP.add)

                rinv = midp.tile([P, 16 * 16], f32, tag="rinv")
                rinv = rinv[:, 0:W * 16]
                nc.scalar.activation(rinv, n2, AF.Abs_reciprocal_sqrt,
                                     bias=eps2[:])

                x8 = x8bufs[ci8 % NX8]
                ci8 += 1
                xt4 = xt.rearrange("p j (b q) -> p j b q", q=4)
                rin4 = rinv.rearrange("p (j b) -> p j b", b=16)
                rin4 = rin4.unsqueeze(3).broadcast_to([P, W, 16, 4])
                x8w = x8[:, 0:W, 0:64].rearrange("p j (b q) -> p j b q", q=4)
                nc.gpsimd.tensor_tensor(x8w, xt4, rin4, OP.mult)

                # ---- one-hot words ----
                oh16 = ohp.tile([P, 16, V // 2], i16, tag="oh16")
                for t in range(W):
                    nc.vector.tensor_scalar(
                        oh16[:, t, :], iota_i[:],
                        s1[:, n_off + t:n_off + t + 1],
                        s2[:, n_off + t:n_off + t + 1],
                        OP.is_equal, OP.mult)
                oh8 = oh16[:].bitcast(f8)  # [P, 16, V]

                # ---- DoubleRow matmuls per pair ----
                for j in range(W // 2):
                    st = x8[:, 2 * j:2 * j + 2, :]
                    first = pair_idx == 0
                    last = pair_idx == NPAIR - 1
                    nc.tensor.matmul(G_ps[:], st, st,
                                     start=first, stop=last,
                                     perf_mode=PM.DoubleRow)
                    mv = oh8[:, 2 * j:2 * j + 2, :]
                    nc.tensor.matmul(S_ps[:, 0:512], st, mv[:, :, 0:512],
                                     start=first, stop=last,
                                     perf_mode=PM.DoubleRow)
                    nc.tensor.matmul(S_ps[:, 512:V], st, mv[:, :, 512:V],
                                     start=first, stop=last,
                                     perf_mode=PM.DoubleRow)
                    pair_idx += 1
                n_off += W

            # ---- tail: assemble cc table [66, V] ----
            for h in (0, 1):
                hs = slice(512 * h, 512 * (h + 1))
                nc.vector.tensor_copy(Sc[0:65, hs], S_ps[0:65, hs])

            # Gram diag -> row vector at partition 0 via ones^T (G o I)
            Gc = epp.tile([64, 64], f32)
            nc.vector.tensor_copy(Gc[:], G_ps[0:64, 0:64])
            gd = epp.tile([64, 64], f32)
            nc.vector.tensor_tensor(gd[:], Gc[:], idn[:], OP.mult)
            sig_ps = psp.tile([1, 64], f32)
            nc.tensor.matmul(sig_ps[:], ones64[:], gd[:],
                             start=True, stop=True)
            nc.vector.tensor_copy(Sc[96:97, 0:64], sig_ps[:])

            for h in (0, 1):
                hs = slice(512 * h, 512 * (h + 1))
                nc.sync.dma_start(cc_in.ap()[:, hs], Sc[:, hs])
            if single:
                nc.sync.dma_start(cc_out.ap(), cc_in.ap())
            else:
                nc.gpsimd.collective_compute(
                    "AllReduce", OP.add,
                    replica_groups=[list(range(N_CORES))],
                    ins=[cc_in.ap().opt()], outs=[cc_out.ap().opt()],
                )

            # ---- epilogue: closed form ----
            R = epp.tile([97, V], f32)
            Rsq = epp.tile([64, V], f16)
            acc0 = epp.tile([64, 1], f32)
            acc1 = epp.tile([64, 1], f32)
            for h, acc in ((0, acc0), (1, acc1)):
                hs = slice(512 * h, 512 * (h + 1))
                nc.sync.dma_start(R[:, hs], cc_out.ap()[:, hs])
                nc.scalar.activation(Rsq[:, hs], R[0:64, hs], AF.Square,
                                     accum_out=acc[:])
            accs = epp.tile([64, 1], f32)
            nc.vector.tensor_tensor(accs[:], acc0[:], acc1[:], OP.add)
            ar_ps = psp.tile([1, 1], f32)
            nc.tensor.matmul(ar_ps[:], ones64[:], accs[:],
                             start=True, stop=True)
            Araw = epp.tile([1, 1], f32)
            nc.vector.tensor_copy(Araw[:], ar_ps[:])

            sig = epp.tile([1, 1], f32)
            nc.vector.tensor_reduce(sig[:], R[96:97, 0:64], AX.X, OP.add)

            csq = epp.tile([1, V], f16)
            sumc2 = epp.tile([1, 1], f32)
            nc.scalar.activation(csq[:], R[64:65, :], AF.Square,
                                 accum_out=sumc2[:])

            num = epp.tile([1, 1], f32)
            nc.vector.tensor_tensor(num[:], Araw[:], sig[:], OP.subtract)
            nc.vector.tensor_scalar(num[:], num[:], 1.0 / 32.0, None, OP.mult)

            pm = epp.tile([1, 1], f32)
            nc.vector.tensor_scalar(pm[:], sumc2[:], NTOK_GLOBAL, None,
                                    OP.subtract)
            nc.vector.tensor_scalar(pm[:], pm[:], 0.5, None, OP.mult)
            denom = epp.tile([1, 1], f32)
            nc.vector.tensor_scalar(denom[:], pm[:], 1.0, None, OP.max)
            maskp = epp.tile([1, 1], f32)
            nc.vector.tensor_scalar(maskp[:], pm[:], 0.0, None, OP.is_gt)
            rden = epp.tile([1, 1], f32)
            nc.vector.reciprocal(rden[:], denom[:])
            lossv = epp.tile([1, 1], f32)
            nc.vector.tensor_tensor(lossv[:], num[:], rden[:], OP.mult)
            nc.vector.tensor_tensor(lossv[:], lossv[:], maskp[:], OP.mult)
            nc.sync.dma_start(loss_dram.ap(), lossv[:])

    nc.compile()
    return nc


def kernel(semantic_state, token_ids):
    from concourse.bass_utils import run_bass_kernel_spmd

    if "nc" not in _cache:
        _cache["nc"] = _build_nc()
    nc = _cache["nc"]

    x = np.ascontiguousarray(np.asarray(semantic_state, dtype=np.float32)
                             ).reshape(N_CORES, TOK_PER_CORE, D)
    t = np.ascontiguousarray(np.asarray(token_ids).astype(np.int32)
                             ).reshape(N_CORES, TOK_PER_CORE)
    in_maps = [{"x": x[c], "tok": t[c]} for c in range(N_CORES)]
    res = run_bass_kernel_spmd(nc, in_maps, core_ids=list(range(N_CORES)),
                               trace=_opts["trace"])
    _cache["last_res"] = res
    out = np.asarray(res.results[0]["loss"], dtype=np.float32)
    return out.reshape(())


# revision 8
# speedup vs baseline: 1.5940x; 1.0028x over previous
"""Trainium2 Bass kernel for nn_BlockContrastiveLoss (fp8 DoubleRow design).

Math: for x in [B*T, 16, 4], x_hat = x / max(||x||_block, eps) per 4-dim
block. Let q = fp8e4m3(x_hat). The pairwise-cosine sum over each vocab
bin is computed EXACTLY for the quantized vectors via

    sum_{i<j in v} q_i . q_j = (||S_v||^2 - sum_{t in v} ||q_t||^2) / 2

so  numerator = (sum_v ||S_v||^2 - sum_t ||q_t||^2) / 32
    P         = (sum_v C_v^2 - N) / 2          (C = global counts)
    loss      = numerator / max(P, 1) * (P > 0)

The only approximation vs the fp32 reference is q != x_hat (measured
rel. err ~1.3e-2 against the jax oracle, within the 2e-2 gate).

Device strategy (8 cores, data-parallel over B*T):
  - partition-major layout: partition p owns tokens p*256+n, n in 0..256.
  - one-hot as packed fp16 words: word w of k-tile t is 56 (fp8 1.0 in
    low byte) if id==2w, 14336 (high byte) if id==2w+1 -- built in ONE
    DVE tensor_scalar (is_equal, mult) with two per-partition scalar
    pointers (s1 = id>>1, s2 = 56 + 14280*(id&1)). Bitcast to fp8 gives
    the full 1024-wide one-hot for 128 tokens in one ~194ns instruction.
  - fp8 DoubleRow matmuls contract 2 k-tiles (256 tokens) at once at
    0.5 cycles/column. Stationary = [q | 1.0 | pad] (80 cols; col 64
    folds the counts row; 65..79 keep preamble zeros to satisfy the
    dual-fp8 Ldweights step%16 rule). Three matmuls per pair:
    Gram [80,80] (trace = sum ||q||^2), S halves [80,512] x2.
  - AllReduce [66,1024]: rows 0..63 = S, 64 = counts, 65[0:64] = Gram
    diag partials; closed-form epilogue on-device; core 0 returns loss.
"""

import numpy as np

N_CORES = 8
B, T, D = 32, 8192, 64
V = 1024
TOK_PER_CORE = B * T // N_CORES      # 32768
P = 128
NPP = TOK_PER_CORE // P              # 256 tokens per partition (k-tiles)
NPAIR = NPP // 2                     # 128 DoubleRow pairs
NTOK_GLOBAL = float(B * T)

_cache = {}
_opts = {"trace": False}


def _build_nc(single=False):
    import concourse.bacc as bacc
    import concourse.mybir as mybir
    import concourse.tile as tile

    dt = mybir.dt
    f32, f16, i32, i16 = dt.float32, dt.float16, dt.int32, dt.int16
    f8 = dt.float8e4
    AF = mybir.ActivationFunctionType
    OP = mybir.AluOpType
    AX = mybir.AxisListType
    PM = mybir.MatmulPerfMode

    nc = bacc.Bacc("TRN2", target_bir_lowering=False, debug=False,
                   num_devices=1 if single else N_CORES)

    x_dram = nc.dram_tensor("x", [TOK_PER_CORE, D], f32, kind="ExternalInput")
    tok_dram = nc.dram_tensor("tok", [TOK_PER_CORE], i32, kind="ExternalInput")
    loss_dram = nc.dram_tensor("loss", [1, 1], f32, kind="ExternalOutput")
    cc_in = nc.dram_tensor("cc_in", [97, V], f32)
    cc_out = nc.dram_tensor("cc_out", [97, V], f32)

    with tile.TileContext(nc) as tc:
        with (
            tc.tile_pool(name="const", bufs=1) as constp,
            tc.tile_pool(name="xin", bufs=4) as xp,
            tc.tile_pool(name="mid", bufs=3) as midp,
            tc.tile_pool(name="oh", bufs=3) as ohp,
            tc.tile_pool(name="psum", bufs=1, space="PSUM") as psp,
            tc.tile_pool(name="ep", bufs=1) as epp,
        ):
            # ---- constants / preamble ----
            iota_i = constp.tile([P, V // 2], i16)
            nc.gpsimd.iota(iota_i[:], pattern=[[1, V // 2]], base=0,
                           channel_multiplier=0)

            ids_i = constp.tile([P, NPP], i32)
            tok_v = tok_dram.ap().rearrange("(p n) -> p n", p=P)
            s1 = constp.tile([P, NPP], f32)     # floor(id/2)
            s2 = constp.tile([P, NPP], f32)     # 56 + 14280*(id&1)
            s1i = constp.tile([P, NPP], i32)
            pari = constp.tile([P, NPP], i32)
            # sliced so the first pairs' scalars are ready quickly
            for a, b in ((0, 16), (16, 64), (64, NPP)):
                sl = slice(a, b)
                nc.sync.dma_start(ids_i[:, sl], tok_v[:, sl])
                nc.vector.tensor_scalar(s1i[:, sl], ids_i[:, sl], 1, None,
                                        OP.logical_shift_right)
                nc.vector.tensor_scalar(pari[:, sl], ids_i[:, sl], 1, None,
                                        OP.bitwise_and)
                nc.gpsimd.tensor_copy(s1[:, sl], s1i[:, sl])
                nc.gpsimd.tensor_copy(s2[:, sl], pari[:, sl])
                nc.gpsimd.tensor_scalar(s2[:, sl], s2[:, sl], 14280.0, None,
                                        OP.mult)
                nc.gpsimd.tensor_scalar(s2[:, sl], s2[:, sl], 56.0, None,
                                        OP.add)

            eps2 = constp.tile([P, 1], f32)
            nc.vector.memset(eps2[:], 1e-12)

            # persistent x8 chunk buffers (manual rotation): [P, 16, 80] fp8
            # cols 64..79 = 1.0 once (col 64 is the counts row; 65..79 land
            # in ignored PSUM rows); the big-mult touches cols 0..63 only.
            # Buffer 0 is set before the loop; 1 and 2 are emitted after
            # chunk 0 so they don't delay the first matmul.
            NX8 = 3
            x8bufs = []
            for i in range(NX8):
                x8b = constp.tile([P, 16, 80], f8, tag=f"x8b{i}")
                x8bufs.append(x8b)
            nc.gpsimd.memset(x8bufs[0][:, :, 64:80], 1.0)

            # tail constants/staging, emitted mid-loop (see below)
            ioc = constp.tile([64, 64], i16)
            idn = constp.tile([64, 64], f32)
            ones64 = constp.tile([64, 1], f32)
            Sc = epp.tile([97, V], f32)

            # PSUM: S rows 0..63 = S, 64 = counts, 65..79 junk
            S_ps = psp.tile([80, V], f32)
            G_ps = psp.tile([80, 80], f32)

            x_n = x_dram.ap().rearrange("(p n) d -> p n d", p=P)

            widths = [2, 2, 4, 8] + [16] * ((NPP - 16) // 16)
            assert sum(widths) == NPP

            n_off = 0
            pair_idx = 0
            ci8 = 0
            for ci, W in enumerate(widths):
                # ---- x-side: load + normalize + quantize ----
                xt = xp.tile([P, 16, D], f32, tag="xt")
                xt = xt[:, 0:W, :]
                nc.sync.dma_start(xt, x_n[:, n_off:n_off + W, :])
                xtf = xt.rearrange("p j d -> p (j d)")

                sq = midp.tile([P, 16 * D], f16, tag="sq")
                sq = sq[:, 0:W * D]
                nc.scalar.activation(sq, xtf, AF.Square)

                sq4 = sq.rearrange("p (t q) -> p t q", q=4)
                t2 = midp.tile([P, 16 * 16, 2], f16, tag="t2")
                t2 = t2[:, 0:W * 16, :]
                nc.vector.tensor_tensor(t2, sq4[:, :, 0:2], sq4[:, :, 2:4],
                                        OP.add)
                n2 = midp.tile([P, 16 * 16], f16, tag="n2")
                n2 = n2[:, 0:W * 16]
                nc.gpsimd.tensor_tensor(n2, t2[:, :, 0], t2[:, :, 1], OP.add)

                rinv = midp.tile([P, 16 * 16], f32, tag="rinv")
                rinv = rinv[:, 0:W * 16]
                nc.scalar.activation(rinv, n2, AF.Abs_reciprocal_sqrt,
                                     bias=eps2[:])

                x8 = x8bufs[ci8 % NX8]
                ci8 += 1
                xt4 = xt.rearrange("p j (b q) -> p j b q", q=4)
                rin4 = rinv.rearrange("p (j b) -> p j b", b=16)
                rin4 = rin4.unsqueeze(3).broadcast_to([P, W, 16, 4])
                x8w = x8[:, 0:W, 0:64].rearrange("p j (b q) -> p j b q", q=4)
                nc.gpsimd.tensor_tensor(x8w, xt4, rin4, OP.mult)

                # ---- one-hot words ----
                oh16 = ohp.tile([P, 16, V // 2], i16, tag="oh16")
                for t in range(W):
                    nc.vector.tensor_scalar(
                        oh16[:, t, :], iota_i[:],
                        s1[:, n_off + t:n_off + t + 1],
                        s2[:, n_off + t:n_off + t + 1],
                        OP.is_equal, OP.mult)
                oh8 = oh16[:].bitcast(f8)  # [P, 16, V]

                # ---- DoubleRow matmuls per pair ----
                for j in range(W // 2):
                    st = x8[:, 2 * j:2 * j + 2, :]
                    first = pair_idx == 0
                    last = pair_idx == NPAIR - 1
                    nc.tensor.matmul(G_ps[:], st, st,
                                     start=first, stop=last,
                                     perf_mode=PM.DoubleRow)
                    mv = oh8[:, 2 * j:2 * j + 2, :]
                    nc.tensor.matmul(S_ps[:, 0:512], st, mv[:, :, 0:512],
                                     start=first, stop=last,
                                     perf_mode=PM.DoubleRow)
                    nc.tensor.matmul(S_ps[:, 512:V], st, mv[:, :, 512:V],
                                     start=first, stop=last,
                                     perf_mode=PM.DoubleRow)
                    pair_idx += 1
                n_off += W
                if ci == 0:
                    nc.gpsimd.memset(x8bufs[1][:, :, 64:80], 1.0)
                    nc.gpsimd.memset(x8bufs[2][:, :, 64:80], 1.0)
                if ci == 5:
                    # cc staging filler rows (partition bases 0/32/64/96)
                    nc.gpsimd.memset(Sc[64:96, :], 0.0)
                    nc.gpsimd.memset(Sc[96:97, :], 0.0)
                    nc.gpsimd.iota(ioc[:], pattern=[[1, 64]], base=0,
                                   channel_multiplier=-1)
                    nc.gpsimd.tensor_scalar(idn[:], ioc[:], 0.0, None,
                                            OP.is_equal)
                    nc.gpsimd.memset(ones64[:], 1.0)

            # ---- tail: assemble cc table, pipelined by halves ----
            # Gram diag -> row vector at partition 0 via ones^T (G o I)
            Gc = epp.tile([64, 64], f32)
            nc.vector.tensor_copy(Gc[:], G_ps[0:64, 0:64])
            gd = epp.tile([64, 64], f32)
            nc.vector.tensor_tensor(gd[:], Gc[:], idn[:], OP.mult)
            sig_ps = psp.tile([1, 64], f32)
            nc.tensor.matmul(sig_ps[:], ones64[:], gd[:],
                             start=True, stop=True)
            nc.vector.tensor_copy(Sc[96:97, 0:64], sig_ps[:])

            for h in (0, 1):
                hs = slice(512 * h, 512 * (h + 1))
                nc.vector.tensor_copy(Sc[0:65, hs], S_ps[0:65, hs])
                nc.sync.dma_start(cc_in.ap()[:, hs], Sc[:, hs])
                if single:
                    nc.sync.dma_start(cc_out.ap()[:, hs], cc_in.ap()[:, hs])
            if single:
                pass
            else:
                nc.gpsimd.collective_compute(
                    "AllReduce", OP.add,
                    replica_groups=[list(range(N_CORES))],
                    ins=[cc_in.ap().opt()], outs=[cc_out.ap().opt()],
                )

            # ---- epilogue: closed form ----
            R = epp.tile([97, V], f32)
            Rsq = epp.tile([64, V], f16)
            acc0 = epp.tile([64, 1], f32)
            acc1 = epp.tile([64, 1], f32)
            for h, acc in ((0, acc0), (1, acc1)):
                hs = slice(512 * h, 512 * (h + 1))
                nc.sync.dma_start(R[:, hs], cc_out.ap()[:, hs])
                nc.scalar.activation(Rsq[:, hs], R[0:64, hs], AF.Square,
                                     accum_out=acc[:])
            accs = epp.tile([64, 1], f32)
            nc.vector.tensor_tensor(accs[:], acc0[:], acc1[:], OP.add)
            ar_ps = psp.tile([1, 1], f32)
            nc.tensor.matmul(ar_ps[:], ones64[:], accs[:],
                             start=True, stop=True)
            Araw = epp.tile([1, 1], f32)
            nc.vector.tensor_copy(Araw[:], ar_ps[:])

            sig = epp.tile([1, 1], f32)
            nc.vector.tensor_reduce(sig[:], R[96:97, 0:64], AX.X, OP.add)

            csq = epp.tile([1, V], f32)
            sumc2 = epp.tile([1, 1], f32)
            nc.vector.tensor_tensor(csq[:], R[64:65, :], R[64:65, :],
                                    OP.mult)
            nc.vector.tensor_reduce(sumc2[:], csq[:], AX.X, OP.add)

            num = epp.tile([1, 1], f32)
            nc.vector.tensor_tensor(num[:], Araw[:], sig[:], OP.subtract)
            nc.vector.tensor_scalar(num[:], num[:], 1.0 / 32.0, None, OP.mult)

            pm = epp.tile([1, 1], f32)
            nc.vector.tensor_scalar(pm[:], sumc2[:], NTOK_GLOBAL, None,
                                    OP.subtract)
            nc.vector.tensor_scalar(pm[:], pm[:], 0.5, None, OP.mult)
            denom = epp.tile([1, 1], f32)
            nc.vector.tensor_scalar(denom[:], pm[:], 1.0, None, OP.max)
            maskp = epp.tile([1, 1], f32)
            nc.vector.tensor_scalar(maskp[:], pm[:], 0.0, None, OP.is_gt)
            rden = epp.tile([1, 1], f32)
            nc.vector.reciprocal(rden[:], denom[:])
            lossv = epp.tile([1, 1], f32)
            nc.vector.tensor_tensor(lossv[:], num[:], rden[:], OP.mult)
            nc.vector.tensor_tensor(lossv[:], lossv[:], maskp[:], OP.mult)
            nc.sync.dma_start(loss_dram.ap(), lossv[:])

    nc.compile()
    return nc


def kernel(semantic_state, token_ids):
    from concourse.bass_utils import run_bass_kernel_spmd

    if "nc" not in _cache:
        _cache["nc"] = _build_nc()
    nc = _cache["nc"]

    x = np.ascontiguousarray(np.asarray(semantic_state, dtype=np.float32)
                             ).reshape(N_CORES, TOK_PER_CORE, D)
    t = np.ascontiguousarray(np.asarray(token_ids).astype(np.int32)
                             ).reshape(N_CORES, TOK_PER_CORE)
    in_maps = [{"x": x[c], "tok": t[c]} for c in range(N_CORES)]
    res = run_bass_kernel_spmd(nc, in_maps, core_ids=list(range(N_CORES)),
                               trace=_opts["trace"])
    _cache["last_res"] = res
    out = np.asarray(res.results[0]["loss"], dtype=np.float32)
    return out.reshape(())
